# revision 5
# baseline (speedup 1.0000x reference)
"""Trainium2 Bass kernel: inclusive cumsum along L for X (4, 8192, 32, 32) f32.

Matmul-cumsum design (8 NeuronCores, SPMD), bf16 I/O:
  - Shard (batch b, L-half h): core i=(2b+h) gets slab X[b, h*4096:(h+1)*4096, :]
    viewed as (4096, 1024), converted to bf16 on the host (2 KiB DRAM rows).
    Each core computes the full cumsum of its slab; the host joins the two
    halves of a batch during unshard with one f32 broadcast add of half-0's
    last row (the sharding seam), after upcasting to f32.
  - Per core, 32 L-blocks of (128, 1024). Local block cumsum = PE matmuls
    with a constant 128x128 upper-triangular-ones bf16 matrix as stationary
    weights in 512-wide chunks (PSUM bank limit): out[m,c] = sum_{k<=m} X[k,c]
    accumulated in f32 PSUM. ACT copies PSUM -> a big SBUF staging tile (loc)
    in bf16; row 127 of each block's local cumsum is the block colsum.
  - Block offsets, per superblock of 8 blocks: one tiny SBUF->SBUF DMA
    gathers the 8 colsums into cm[9,1024] = [carry; colsum_0..7]. For each
    block, one K=9 matmul with lhsT = triA_rep[:, b*128:(b+1)*128] (the
    offset-selector column for block b replicated 128x) emits that block's
    offset row replicated across all 128 partitions, straight into f32 PSUM
    -- offsets are computed AND partition-broadcast in a single cheap matmul.
    A K=9,M=1 matmul emits the next superblock carry; DVE converts it to
    cm[0] of superblock s+1.
  - Final: DVE tensor_tensor add (in0 = loc bf16 SBUF, in1 = replicated
    offsets f32 PSUM) -> yt bf16, DMA out 2 blocks (512 KiB) at a time.
  - Measured ~90 us on 8 cores (baseline transpose+scan design: 112 us).
    Engine profile: PE ~65 us (the instruction-stream backbone; the chip
    power-throttles PE to ~1.1 ns/col), DMA ~59 us, DVE ~39 us, ACT ~36 us.
"""

import numpy as np
import ml_dtypes
from contextlib import ExitStack

import concourse.bass as bass
import concourse.tile as tile
from concourse import bacc, mybir
from concourse.bass_utils import run_bass_kernel_spmd

BF16 = ml_dtypes.bfloat16

N_CORES = 8
B, L, D, N = 4, 8192, 32, 32
C = D * N               # 1024 columns
LH = L // 2             # 4096 rows per core (L-half)
P = 128                 # partitions / L-block rows
NBLK = LH // P          # 32 L-blocks per core
SB = 8                  # blocks per superblock (offset batch)
NSUP = NBLK // SB       # 4 superblocks per core
GRP = 2                 # L-blocks per DMA (in and out)
CH = C // 512           # 512-wide matmul chunks per block

_CACHE = {}


def _build_program():
    f32 = mybir.dt.float32
    bf16 = mybir.dt.bfloat16
    nc = bacc.Bacc(
        trn_type="TRN2", debug=False, num_devices=N_CORES, num_swdge_queues=2
    )
    x = nc.dram_tensor("x", [LH, C], bf16, kind="ExternalInput").ap()
    tri = nc.dram_tensor("tri", [P, P], bf16, kind="ExternalInput").ap()
    triar = nc.dram_tensor("triar", [SB + 1, SB * P], bf16, kind="ExternalInput").ap()
    triac = nc.dram_tensor("triac", [SB + 1, 1], bf16, kind="ExternalInput").ap()
    y = nc.dram_tensor("y", [LH, C], bf16, kind="ExternalOutput").ap()

    with tile.TileContext(nc) as tc, ExitStack() as ctx:
        const_pool = ctx.enter_context(tc.tile_pool(name="const", bufs=1))
        xin_pool = ctx.enter_context(tc.tile_pool(name="xin", bufs=4))
        yout_pool = ctx.enter_context(tc.tile_pool(name="yout", bufs=4))
        cmat_pool = ctx.enter_context(tc.tile_pool(name="cmat", bufs=2))
        mmps_pool = ctx.enter_context(tc.tile_pool(name="mmps", bufs=2, space="PSUM"))
        auxps_pool = ctx.enter_context(tc.tile_pool(name="auxps", bufs=2, space="PSUM"))

        tri_sb = const_pool.tile([P, P], bf16, name="tri_sb")
        triar_sb = const_pool.tile([SB + 1, SB * P], bf16, name="triar_sb")
        triac_sb = const_pool.tile([SB + 1, 1], bf16, name="triac_sb")
        nc.sync.dma_start(out=tri_sb[:], in_=tri)
        nc.sync.dma_start(out=triar_sb[:], in_=triar)
        nc.sync.dma_start(out=triac_sb[:], in_=triac)

        loc = const_pool.tile([P, NBLK * C], bf16, name="loc")  # local cumsums

        cmats = []
        for s in range(NSUP):
            cm = cmat_pool.tile([SB + 1, C], bf16, name=f"cm{s}", tag="cm", bufs=2)
            cmats.append(cm)
            if s == 0:
                nc.gpsimd.memset(cm[0:1, :], 0.0)

        def mm1_group(s, g):
            blk0 = s * SB + g * GRP
            xt = xin_pool.tile([P, GRP * C], bf16, name="xt", tag="xt", bufs=4)
            src = x[blk0 * P : (blk0 + GRP) * P, :].rearrange(
                "(ks p) c -> p ks c", p=P
            )
            nc.sync.dma_start(out=xt[:].rearrange("p (ks c) -> p ks c", ks=GRP),
                              in_=src)
            for ks in range(GRP):
                b = blk0 + ks
                ps = mmps_pool.tile([P, C], f32, name="ps", tag="ps", bufs=2)
                for ch in range(CH):
                    nc.tensor.matmul(
                        out=ps[:, ch * 512 : (ch + 1) * 512],
                        lhsT=tri_sb[:],
                        rhs=xt[:, ks * C + ch * 512 : ks * C + (ch + 1) * 512],
                    )
                nc.scalar.copy(loc[:, b * C : (b + 1) * C], ps[:])

        def finish_group(s, g):
            # offsets for blocks bs=g*GRP..g*GRP+1; bs<4 needs cm rows 0..4 only
            cm = cmats[s]
            blk0 = s * SB + g * GRP
            yt = yout_pool.tile([P, GRP * C], bf16, name="yt", tag="yt", bufs=4)
            for ks in range(GRP):
                b = blk0 + ks
                bs = b - s * SB
                kk = 5 if bs < 4 else SB + 1
                rp = auxps_pool.tile([P, C], f32, name="rp", tag="rp", bufs=2)
                for ch in range(CH):
                    nc.tensor.matmul(
                        out=rp[:, ch * 512 : (ch + 1) * 512],
                        lhsT=triar_sb[0:kk, bs * P : (bs + 1) * P],
                        rhs=cm[0:kk, ch * 512 : (ch + 1) * 512],
                    )
                nc.vector.tensor_tensor(
                    out=yt[:, ks * C : (ks + 1) * C],
                    in0=loc[:, b * C : (b + 1) * C],
                    in1=rp[:],
                    op=mybir.AluOpType.add,
                )
            ydst = y[blk0 * P : (blk0 + GRP) * P, :].rearrange(
                "(ks p) c -> p ks c", p=P
            )
            out_eng = nc.gpsimd if g % 2 == 0 else nc.scalar
            out_eng.dma_start(
                out=ydst, in_=yt[:].rearrange("p (ks c) -> p ks c", ks=GRP)
            )

        def gather_half(s, half):
            cm = cmats[s]
            lo = s * SB * C + half * 4 * C
            nc.scalar.dma_start(
                out=cm[1 + half * 4 : 5 + half * 4, :],
                in_=loc[P - 1 : P, lo : lo + 4 * C].rearrange(
                    "one (m c) -> one m c", m=4
                ),
            )

        for s in range(NSUP):
            mm1_group(s, 0)
            mm1_group(s, 1)
            gather_half(s, 0)       # cm rows 1..4 after blocks 0..3 copied
            mm1_group(s, 2)
            finish_group(s, 0)      # blocks 0,1 (K=5)
            finish_group(s, 1)      # blocks 2,3 (K=5)
            mm1_group(s, 3)
            gather_half(s, 1)       # cm rows 5..8
            if s + 1 < NSUP:
                cp = auxps_pool.tile([1, C], f32, name="cp", tag="rp", bufs=2)
                for ch in range(CH):
                    nc.tensor.matmul(
                        out=cp[:, ch * 512 : (ch + 1) * 512],
                        lhsT=triac_sb[:],
                        rhs=cmats[s][:, ch * 512 : (ch + 1) * 512],
                    )
                nc.vector.tensor_copy(cmats[s + 1][0:1, :], cp[:])
            finish_group(s, 2)      # blocks 4,5 (K=9)
            finish_group(s, 3)      # blocks 6,7 (K=9)

    nc.compile()
    return nc


def _get_program():
    if "nc" not in _CACHE:
        _CACHE["nc"] = _build_program()
    return _CACHE["nc"]


def _consts():
    tri = np.triu(np.ones((P, P), np.float32)).astype(BF16)  # tri[k,m]=1 for k<=m
    # triA[k, b]: offset selector for block b: carry (k=0) + colsum_a (k=1+a, a<b)
    tria = np.zeros((SB + 1, SB + 1), np.float32)
    tria[0, :] = 1.0
    for a in range(SB):
        tria[1 + a, a + 1 :] = 1.0
    triar = np.repeat(tria[:, :SB], P, axis=1).astype(BF16)   # [9, 8*128]
    triac = np.ones((SB + 1, 1), np.float32).astype(BF16)     # next carry selector
    return tri, triar, triac


def kernel(X_in, _trace=False, _tmpdir=None, _trace_cores=None):
    X = np.asarray(X_in, dtype=np.float32)
    assert X.shape == (B, L, D, N), X.shape
    Xv = X.reshape(B, L, C)
    tri, triar, triac = _consts()
    nc = _get_program()
    in_maps = []
    for i in range(N_CORES):
        b, h = i // 2, i % 2
        slab = np.ascontiguousarray(Xv[b, h * LH : (h + 1) * LH, :]).astype(BF16)
        in_maps.append({"x": slab, "tri": tri, "triar": triar, "triac": triac})
    kwargs = {}
    if _trace:
        kwargs = dict(
            trace=True,
            tmpdir=_tmpdir,
            trace_cores=_trace_cores or list(range(N_CORES)),
        )
    res = run_bass_kernel_spmd(nc, in_maps, core_ids=list(range(N_CORES)), **kwargs)
    out = np.empty((B, L, C), np.float32)
    for i in range(N_CORES):
        b, h = i // 2, i % 2
        out[b, h * LH : (h + 1) * LH, :] = res.results[i]["y"].astype(np.float32)
    for b in range(B):
        out[b, LH:, :] += out[b, LH - 1 : LH, :]
    kernel.last_results = res
    return out.reshape(B, L, D, N)


# revision 7
# speedup vs baseline: 1.0980x; 1.0980x over previous
"""Trainium2 Bass kernel: inclusive cumsum along L for X (4, 8192, 32, 32) f32.

Matmul-cumsum design (8 NeuronCores, SPMD), bf16 I/O:
  - Shard (batch b, L-half h): core i=(2b+h) gets slab X[b, h*4096:(h+1)*4096, :]
    viewed as (4096, 1024), converted to bf16 on the host (2 KiB DRAM rows).
    Each core computes the full cumsum of its slab; the host joins the two
    halves of a batch during unshard with one f32 broadcast add of half-0's
    last row (the sharding seam), after upcasting to f32.
  - Per core, 32 L-blocks of (128, 1024). Local block cumsum = PE matmuls
    with a constant 128x128 upper-triangular-ones bf16 matrix as stationary
    weights in 512-wide chunks (PSUM bank limit): out[m,c] = sum_{k<=m} X[k,c]
    accumulated in f32 PSUM. ACT copies PSUM -> a big SBUF staging tile (loc)
    in bf16; row 127 of each block's local cumsum is the block colsum.
  - Block offsets, per superblock of 8 blocks: one tiny SBUF->SBUF DMA
    gathers the colsums into cm[SB+1,1024] = [carry; colsum_0..7]. For each
    block, one K=9 matmul with lhsT = triA_rep[:, b*128:(b+1)*128] (the
    offset-selector column for block b replicated 128x) emits that block's
    offset row replicated across all 128 partitions, straight into f32 PSUM
    -- offsets are computed AND partition-broadcast in a single cheap matmul.
    A K=9,M=1 matmul emits the next superblock carry; DVE converts it to
    cm[0] of superblock s+1.
  - Final: DVE tensor_tensor add (in0 = loc bf16 SBUF, in1 = replicated
    offsets f32 PSUM) -> yt bf16, DMA out 2 blocks (512 KiB) at a time.
  - Measured ~90 us on 8 cores (baseline transpose+scan design: 112 us).
    Engine profile: PE ~65 us (the instruction-stream backbone; the chip
    power-throttles PE to ~1.1 ns/col), DMA ~59 us, DVE ~39 us, ACT ~36 us.
"""

import numpy as np
import ml_dtypes
from contextlib import ExitStack

import concourse.bass as bass
import concourse.tile as tile
from concourse import bacc, mybir
from concourse.bass_utils import run_bass_kernel_spmd

BF16 = ml_dtypes.bfloat16

N_CORES = 8
B, L, D, N = 4, 8192, 32, 32
C = D * N               # 1024 columns
LH = L // 2             # 4096 rows per core (L-half)
P = 128                 # partitions / L-block rows
NBLK = LH // P          # 32 L-blocks per core
SB = 4                  # blocks per superblock (offset batch)
NSUP = NBLK // SB       # 4 superblocks per core
GRP = 2                 # L-blocks per DMA (in and out)
CH = C // 512           # 512-wide matmul chunks per block

_CACHE = {}


def _build_program():
    f32 = mybir.dt.float32
    bf16 = mybir.dt.bfloat16
    nc = bacc.Bacc(
        trn_type="TRN2", debug=False, num_devices=N_CORES, num_swdge_queues=2
    )
    x = nc.dram_tensor("x", [LH, C], bf16, kind="ExternalInput").ap()
    tri = nc.dram_tensor("tri", [P, P], bf16, kind="ExternalInput").ap()
    triar = nc.dram_tensor("triar", [SB + 1, SB * P], bf16, kind="ExternalInput").ap()
    triac = nc.dram_tensor("triac", [SB + 1, 1], bf16, kind="ExternalInput").ap()
    y = nc.dram_tensor("y", [LH, C], bf16, kind="ExternalOutput").ap()

    with tile.TileContext(nc) as tc, ExitStack() as ctx:
        const_pool = ctx.enter_context(tc.tile_pool(name="const", bufs=1))
        xin_pool = ctx.enter_context(tc.tile_pool(name="xin", bufs=4))
        yout_pool = ctx.enter_context(tc.tile_pool(name="yout", bufs=4))
        cmat_pool = ctx.enter_context(tc.tile_pool(name="cmat", bufs=2))
        mmps_pool = ctx.enter_context(tc.tile_pool(name="mmps", bufs=2, space="PSUM"))
        auxps_pool = ctx.enter_context(tc.tile_pool(name="auxps", bufs=2, space="PSUM"))

        tri_sb = const_pool.tile([P, P], bf16, name="tri_sb")
        triar_sb = const_pool.tile([SB + 1, SB * P], bf16, name="triar_sb")
        triac_sb = const_pool.tile([SB + 1, 1], bf16, name="triac_sb")
        nc.sync.dma_start(out=tri_sb[:], in_=tri)
        nc.sync.dma_start(out=triar_sb[:], in_=triar)
        nc.sync.dma_start(out=triac_sb[:], in_=triac)

        loc = const_pool.tile([P, NBLK * C], bf16, name="loc")  # local cumsums

        cmats = []
        for s in range(NSUP):
            cm = cmat_pool.tile([SB + 1, C], bf16, name=f"cm{s}", tag="cm", bufs=2)
            cmats.append(cm)
            if s == 0:
                nc.gpsimd.memset(cm[0:1, :], 0.0)

        for s in range(NSUP):
            # ---- local cumsums for this superblock ----
            for g in range(SB // GRP):
                blk0 = s * SB + g * GRP
                xt = xin_pool.tile([P, GRP * C], bf16, name="xt", tag="xt", bufs=4)
                src = x[blk0 * P : (blk0 + GRP) * P, :].rearrange(
                    "(ks p) c -> p ks c", p=P
                )
                nc.sync.dma_start(out=xt[:].rearrange("p (ks c) -> p ks c", ks=GRP),
                                  in_=src)
                for ks in range(GRP):
                    b = blk0 + ks
                    ps = mmps_pool.tile([P, C], f32, name="ps", tag="ps", bufs=2)
                    for ch in range(CH):
                        nc.tensor.matmul(
                            out=ps[:, ch * 512 : (ch + 1) * 512],
                            lhsT=tri_sb[:],
                            rhs=xt[:, ks * C + ch * 512 : ks * C + (ch + 1) * 512],
                        )
                    nc.scalar.copy(loc[:, b * C : (b + 1) * C], ps[:])

            # ---- colsum gather + carry for this superblock ----
            cm = cmats[s]
            nc.gpsimd.dma_start(
                out=cm[1 : SB + 1, :],
                in_=loc[P - 1 : P, s * SB * C : (s + 1) * SB * C].rearrange(
                    "one (m c) -> one m c", m=SB
                ),
            )
            if s + 1 < NSUP:
                cp = auxps_pool.tile([1, C], f32, name="cp", tag="rp", bufs=2)
                for ch in range(CH):
                    nc.tensor.matmul(
                        out=cp[:, ch * 512 : (ch + 1) * 512],
                        lhsT=triac_sb[:],
                        rhs=cm[:, ch * 512 : (ch + 1) * 512],
                    )
                nc.vector.tensor_copy(cmats[s + 1][0:1, :], cp[:])

            # ---- replicated-offset matmuls + final adds + DMA out ----
            for g in range(SB // GRP):
                blk0 = s * SB + g * GRP
                yt = yout_pool.tile([P, GRP * C], bf16, name="yt", tag="yt", bufs=4)
                for ks in range(GRP):
                    b = blk0 + ks
                    bs = b - s * SB
                    rp = auxps_pool.tile([P, C], f32, name="rp", tag="rp", bufs=2)
                    for ch in range(CH):
                        nc.tensor.matmul(
                            out=rp[:, ch * 512 : (ch + 1) * 512],
                            lhsT=triar_sb[:, bs * P : (bs + 1) * P],
                            rhs=cm[:, ch * 512 : (ch + 1) * 512],
                        )
                    nc.vector.tensor_tensor(
                        out=yt[:, ks * C : (ks + 1) * C],
                        in0=loc[:, b * C : (b + 1) * C],
                        in1=rp[:],
                        op=mybir.AluOpType.add,
                    )
                ydst = y[blk0 * P : (blk0 + GRP) * P, :].rearrange(
                    "(ks p) c -> p ks c", p=P
                )
                out_eng = nc.gpsimd if g % 2 == 0 else nc.sync
                out_eng.dma_start(
                    out=ydst, in_=yt[:].rearrange("p (ks c) -> p ks c", ks=GRP)
                )

    nc.compile()
    return nc


def _get_program():
    if "nc" not in _CACHE:
        _CACHE["nc"] = _build_program()
    return _CACHE["nc"]


def _consts():
    tri = np.triu(np.ones((P, P), np.float32)).astype(BF16)  # tri[k,m]=1 for k<=m
    # triA[k, b]: offset selector for block b: carry (k=0) + colsum_a (k=1+a, a<b)
    tria = np.zeros((SB + 1, SB + 1), np.float32)
    tria[0, :] = 1.0
    for a in range(SB):
        tria[1 + a, a + 1 :] = 1.0
    triar = np.repeat(tria[:, :SB], P, axis=1).astype(BF16)   # [9, 8*128]
    triac = np.ones((SB + 1, 1), np.float32).astype(BF16)     # next carry selector
    return tri, triar, triac


def kernel(X_in, _trace=False, _tmpdir=None, _trace_cores=None):
    X = np.asarray(X_in, dtype=np.float32)
    assert X.shape == (B, L, D, N), X.shape
    Xv = X.reshape(B, L, C)
    tri, triar, triac = _consts()
    nc = _get_program()
    in_maps = []
    for i in range(N_CORES):
        b, h = i // 2, i % 2
        slab = np.ascontiguousarray(Xv[b, h * LH : (h + 1) * LH, :]).astype(BF16)
        in_maps.append({"x": slab, "tri": tri, "triar": triar, "triac": triac})
    kwargs = {}
    if _trace:
        kwargs = dict(
            trace=True,
            tmpdir=_tmpdir,
            trace_cores=_trace_cores or list(range(N_CORES)),
        )
    res = run_bass_kernel_spmd(nc, in_maps, core_ids=list(range(N_CORES)), **kwargs)
    out = np.empty((B, L, C), np.float32)
    for i in range(N_CORES):
        b, h = i // 2, i % 2
        out[b, h * LH : (h + 1) * LH, :] = res.results[i]["y"].astype(np.float32)
    for b in range(B):
        out[b, LH:, :] += out[b, LH - 1 : LH, :]
    kernel.last_results = res
    return out.reshape(B, L, D, N)
